# revision 30
# baseline (speedup 1.0000x reference)
"""MHA forward kernel for Trainium2 (Bass/Tile), sharded over (batch, head)
pairs across 8 NeuronCores.

Math (per (b,h) pair):
    scores = softmax(Q K^T / sqrt(64) + bias)   # bias broadcast over (b,h)
    out    = scores @ V

The whole matmul pipeline runs in the PE's 64x128 row-tiled mode
(tile_position (0,0) / (64,0)), exploiting the D=64 contraction:
  - MM1 computes TWO score k-tiles concurrently (even k-tiles through array
    rows 0-63, odd through rows 64-127), doubling QK^T throughput.
  - bias is added into PSUM via identity matmuls for N_PE of the 8 chunks
    per q-tile (PE-path), via DVE scalar_tensor_tensor for the rest
    (DVE-path) -- the split balances PE vs DVE vs ACT occupancy. Each PSUM
    bank is only ever written by its own PE tile (two tiles writing one
    bank wedges the device); the cross-half bias rows come from bias_x, a
    partition-swapped host-side copy, through sident (a swapped identity).
  - MM2 accumulates the lower 64 rows of every k-tile into o_lo (tile T0)
    and the upper rows into o_hi (T8) concurrently; the epilogue reduces
    them, PE-transposes 128-row blocks, and divides by the ones-column sum.
  - epilogue transpose/div/store steps are deferred and spread one per
    stream iteration so their DVE latency never head-of-line-blocks the
    in-order PE queue; transpose PSUM targets share the score pool's ring.
  - bias q-slices stream in lazily (2 q-tiles ahead) to keep the startup
    DMA window clear for the first pair's Q/K/V.
Q is pre-scaled by 1/8 host-side (exact in bf16) and duplicated onto both
partition halves; K is host-packed even/odd so each PE half-array reads its
stationary tiles from its own SBUF partitions.
"""

import os
import sys

import numpy as np

for _p in ("/opt/trn_rl_repo",):
    if _p not in sys.path and os.path.isdir(_p):
        sys.path.insert(0, _p)

B, H, S, D = 2, 16, 2048, 64
N_CORES = 8
PAIRS = B * H                     # 32
PPC = PAIRS // N_CORES            # 4 pairs per core
SCALE = 1.0 / 8.0                 # 1/sqrt(64)

KT = S // 128                     # 16 k-tiles of 128
KP = KT // 2                      # 8 k-tile pairs (even/odd)
QTILE = 512
QT = S // QTILE                   # 4 q-tiles
PP_BUFS = int(os.environ.get("PP_BUFS", "4"))
PD_BUFS = int(os.environ.get("PD_BUFS", "4"))
SC_BUFS = int(os.environ.get("SC_BUFS", "3"))
LAG = int(os.environ.get("LAG", "3"))
EPI_BUFS = int(os.environ.get("EPI_BUFS", "3"))
N_PE = int(os.environ.get("N_PE", "4"))

_CACHE = {}


def _pe_pattern():
    """Which of the 8 chunks per q-tile add bias on the PE (identity
    matmuls) vs the DVE (scalar_tensor_tensor)."""
    pat = [False] * KP
    if N_PE >= KP:
        return [True] * KP
    # grouped pairs: PE,PE,DVE,DVE,... so adjacent DVE chunks can share
    # one batched exp instruction
    order = [0, 1, 4, 5, 2, 3, 6, 7]
    for i in range(N_PE):
        pat[order[i]] = True
    return pat


def _build_nc():
    import concourse.mybir as mybir
    import concourse.tile as tile
    from concourse import bacc

    f32 = mybir.dt.float32
    bf16 = mybir.dt.bfloat16
    nc = bacc.Bacc(None)

    qd = nc.declare_dram_parameter("qd", [PPC, 128, S], bf16, isOutput=False)
    kp = nc.declare_dram_parameter("kp", [PPC, 128, KP, 128], bf16, isOutput=False)
    v1 = nc.declare_dram_parameter("v1", [PPC, 128, KT, D + 1], bf16, isOutput=False)
    biasT = nc.declare_dram_parameter("biasT", [S, S], bf16, isOutput=False)
    ident_d = nc.declare_dram_parameter("ident", [128, 128], bf16, isOutput=False)
    out = nc.declare_dram_parameter("out", [PPC, S, D], bf16, isOutput=True)

    pe_path = _pe_pattern()
    n_pe = sum(pe_path)
    # bias_x holds the partition-swapped bias rows for PE-path chunks so
    # each PSUM bank's bias is added by the SAME tile that wrote its
    # scores (two tiles writing one bank hangs the device):
    #   partitions 0-63:  rows 64-127 of each PE-chunk's even k-tile
    #   partitions 64-127: rows 0-63 of each PE-chunk's odd k-tile
    if n_pe:
        bias_x = nc.declare_dram_parameter(
            "bias_x", [128, n_pe, S], bf16, isOutput=False
        )

    with tile.TileContext(nc) as tc:
        with (
            tc.tile_pool(name="const", bufs=1) as const_pool,
            tc.tile_pool(name="bias", bufs=1) as bias_pool,
            tc.tile_pool(name="qk", bufs=2) as qk_pool,
            tc.tile_pool(name="vv", bufs=2) as v_pool,
            tc.tile_pool(name="probP", bufs=PP_BUFS) as pP_pool,
            tc.tile_pool(name="probD", bufs=PD_BUFS) as pD_pool,
            tc.tile_pool(name="epi", bufs=EPI_BUFS) as epi_pool,
            tc.tile_pool(name="sc", bufs=SC_BUFS, space="PSUM") as sc_pool,
            tc.tile_pool(name="acc", bufs=1, space="PSUM") as acc_pool,
        ):
            ident = const_pool.tile([128, 128], bf16)
            nc.sync.dma_start(ident[:], ident_d[:])
            # Dummy matmuls bridge the startup DMA window: they keep the
            # PE HAM activity monitor at full clock (idle >3.4us would
            # re-throttle to 1.2GHz) and cost nothing -- the PE would be
            # idle waiting for Q/K/bias anyway. Output is never read.
            warm = sc_pool.tile([128, 2, QTILE], f32, tag="sc", name="warm")
            for w in range(48):
                nc.tensor.matmul(
                    warm[:, 0, 0:128], ident[0:64, :], ident[0:64, :],
                    start=(w == 0), stop=(w == 47), tile_position=(0, 0),
                )
            # partition-swapped identity: rows 0-63 hold e_{64+r}, rows
            # 64-127 hold e_r -- lets each tile add bias into the
            # opposite 64-row half of its own PSUM bank.
            sident = const_pool.tile([128, 128], bf16)
            nc.sync.dma_start(sident[0:64, :], ident_d[64:128, :])
            nc.sync.dma_start(sident[64:128, :], ident_d[0:64, :])

            def load_pair(p):
                qd_sb = qk_pool.tile([128, S], bf16, tag="q")
                nc.sync.dma_start(qd_sb[:], qd[p])
                kp_sb = qk_pool.tile([128, KP, 128], bf16, tag="k")
                nc.sync.dma_start(kp_sb[:], kp[p])
                # V with ones-column appended (host-side): second matmul
                # also yields sum(exp) in output row D.
                # host pre-arranged to the SBUF layout: the DMA moves
                # one contiguous 2080B run per partition instead of 16
                # 130B runs (16x fewer descriptors -- less startup DMA
                # engine congestion)
                v_sb = v_pool.tile([128, KT, D + 1], bf16)
                nc.sync.dma_start(v_sb[:], v1[p])
                return qd_sb, kp_sb, v_sb

            # pair 0 loads first so MM1 can start before the bias stream.
            loaded = {0: load_pair(0)}

            # Full bias^T resident in SBUF: [128, KT, S] bf16. Only the
            # qt=0 slice is loaded upfront (per-ktile, chunk order);
            # later q-slices are issued lazily from the stream loop so
            # the startup DMA window isn't congested by 10MB of bias.
            bias_sb = bias_pool.tile([128, KT, S], bf16)
            bias_src = biasT.rearrange("(kt p) q -> p kt q", p=128)
            if n_pe:
                bias_x_sb = bias_pool.tile([128, n_pe, S], bf16, tag="bx")

            def load_bias_slice(qc, fine):
                qsl = slice(qc * QTILE, (qc + 1) * QTILE)
                if n_pe:
                    nc.sync.dma_start(
                        bias_x_sb[:, :, qsl], bias_x[:, :, qsl]
                    )
                if fine:
                    for kt in range(KT):
                        nc.sync.dma_start(
                            bias_sb[:, kt, qsl], bias_src[:, kt, qsl]
                        )
                else:
                    nc.sync.dma_start(
                        bias_sb[:, :, qsl], bias_src[:, :, qsl]
                    )

            load_bias_slice(0, fine=True)

            pe_idx = {}
            for j in range(KP):
                if pe_path[j]:
                    pe_idx[j] = len(pe_idx)

            stream = []  # (p, qt, j, is_last_of_qt)
            for p in range(PPC):
                for qt in range(QT):
                    for j in range(KP):
                        stream.append((p, qt, j, j == KP - 1))

            state = {}  # (p, qt) -> dict with o_ev, o_od
            dve_hold = [None]

            def produce(p, qt, j):
                qd_sb, kp_sb, _ = loaded[p]
                qs_lo = qd_sb[0:64, qt * QTILE : (qt + 1) * QTILE]
                qs_hi = qd_sb[64:128, qt * QTILE : (qt + 1) * QTILE]
                kt_e, kt_o = 2 * j, 2 * j + 1
                qsl = slice(qt * QTILE, (qt + 1) * QTILE)
                on_pe = pe_path[j]
                s = sc_pool.tile([128, 2, QTILE], f32, tag="sc")
                # scores: even k-tile through rows 0-63 (T0), odd through
                # rows 64-127 (T8) -- concurrent in 64x128 tiled mode.
                nc.tensor.matmul(
                    s[:, 0, :], kp_sb[0:64, j, :], qs_lo,
                    start=True, stop=not on_pe, tile_position=(0, 0),
                )
                nc.tensor.matmul(
                    s[:, 1, :], kp_sb[64:128, j, :], qs_hi,
                    start=True, stop=not on_pe, tile_position=(64, 0),
                )
                if on_pe:
                    # bias via identity matmuls, every bank written ONLY
                    # by its own tile: T0 owns bank0 (even k-tile), T8
                    # owns bank1 (odd). The cross-half rows come from
                    # bias_x (partition-swapped copy) through sident.
                    c = pe_idx[j]
                    nc.tensor.matmul(
                        s[:, 0, :], ident[0:64, :], bias_sb[0:64, kt_e, qsl],
                        start=False, stop=False, tile_position=(0, 0),
                    )
                    nc.tensor.matmul(
                        s[:, 1, :], ident[64:128, :], bias_sb[64:128, kt_o, qsl],
                        start=False, stop=False, tile_position=(64, 0),
                    )
                    nc.tensor.matmul(
                        s[:, 0, :], sident[0:64, :], bias_x_sb[0:64, c, qsl],
                        start=False, stop=True, tile_position=(0, 0),
                    )
                    nc.tensor.matmul(
                        s[:, 1, :], sident[64:128, :], bias_x_sb[64:128, c, qsl],
                        start=False, stop=True, tile_position=(64, 0),
                    )
                    p_sb = pP_pool.tile([128, 2, QTILE], bf16, tag="pP")
                    nc.scalar.activation(
                        p_sb[:], s[:], mybir.ActivationFunctionType.Exp
                    )
                    return (p_sb, 0)
                p_sb = pD_pool.tile([128, 2, QTILE], bf16, tag="pD")
                nc.vector.scalar_tensor_tensor(
                    p_sb[:],
                    s[:],
                    1.0,
                    bias_sb[:, kt_e : kt_e + 2, qsl],
                    op0=mybir.AluOpType.mult,
                    op1=mybir.AluOpType.add,
                )
                nc.scalar.activation(
                    p_sb[:], p_sb[:], mybir.ActivationFunctionType.Exp
                )
                return (p_sb, 0)

            def consume(p, qt, j, p_sb_ref):
                p_sb, base = p_sb_ref
                _, _, v_sb = loaded[p]
                st = state[(p, qt)]
                kt_e, kt_o = 2 * j, 2 * j + 1
                # T0 accumulates the lower 64 rows of every k-tile into
                # o_lo; T8 the upper 64 rows into o_hi -- concurrent.
                nc.tensor.matmul(
                    st["o_lo"],
                    v_sb[0:64, kt_e, :],
                    p_sb[0:64, base + 0, :],
                    start=(j == 0), stop=False,
                    tile_position=(0, 0),
                )
                nc.tensor.matmul(
                    st["o_hi"],
                    v_sb[64:128, kt_e, :],
                    p_sb[64:128, base + 0, :],
                    start=(j == 0), stop=False,
                    tile_position=(64, 0),
                )
                nc.tensor.matmul(
                    st["o_lo"],
                    v_sb[0:64, kt_o, :],
                    p_sb[0:64, base + 1, :],
                    start=False, stop=(j == KP - 1),
                    tile_position=(0, 0),
                )
                nc.tensor.matmul(
                    st["o_hi"],
                    v_sb[64:128, kt_o, :],
                    p_sb[64:128, base + 1, :],
                    start=False, stop=(j == KP - 1),
                    tile_position=(64, 0),
                )

            epi_steps = []  # deferred per-128-row epilogue work items
            iter_box = [0]

            def epilogue_start(p, qt):
                # Only the PSUM->SBUF reduce runs immediately (frees
                # o_acc); the transpose/div/store steps are spread one
                # per stream iteration so their DVE latency never
                # head-of-line-blocks the in-order PE queue.
                st = state.pop((p, qt))
                # bf16 here halves the PE transpose cost (1 cyc/row vs 2
                # for fp32); the 2-element add + later divide tolerate it
                o_sb = epi_pool.tile([D + 1, QTILE], bf16, tag="osb")
                with nc.allow_low_precision("bf16 epilogue, tol 2e-2"):
                    nc.vector.tensor_reduce(
                        o_sb[:],
                        st["o_acc"][:].rearrange("p a b -> p b a"),
                        axis=mybir.AxisListType.X,
                        op=mybir.AluOpType.add,
                    )
                for c in range(4):
                    # first transpose waits the DVE reduce (~1.2us); hold
                    # it back two stream slots so the in-order PE queue
                    # keeps streaming matmuls meanwhile
                    epi_steps.append((iter_box[0] + 2 + c, p, qt, c, o_sb))

            def epi_step():
                if not epi_steps or epi_steps[0][0] > iter_box[0]:
                    return
                _, p, qt, c, o_sb = epi_steps.pop(0)
                # transpose target shares the score pool's ring (same
                # shape/tag keeps one 3-buf ring within 6 PSUM banks)
                t_tile = sc_pool.tile([128, 2, QTILE], bf16, tag="sc")
                t_psum = t_tile[:, 0, : D + 1]
                nc.tensor.transpose(
                    t_psum,
                    o_sb[:, c * 128 : (c + 1) * 128],
                    ident[: D + 1, : D + 1],
                )
                r_sb = epi_pool.tile([128, 1], f32, tag="rsb")
                nc.vector.reciprocal(r_sb[:], t_tile[:, 0, D : D + 1])
                f_sb = epi_pool.tile([128, D], bf16, tag="fsb")
                nc.vector.tensor_scalar_mul(f_sb[:], t_tile[:, 0, :D], r_sb[:])
                row0 = qt * QTILE + c * 128
                nc.sync.dma_start(out[p, row0 : row0 + 128, :], f_sb[:])

            bias_loaded = {0}

            def want_bias(qc):
                if 0 <= qc < QT and qc not in bias_loaded:
                    bias_loaded.add(qc)
                    load_bias_slice(qc, fine=False)

            pending = []  # (p, qt, j, p_sb, is_last)
            for i, (p, qt, j, is_last) in enumerate(stream):
                iter_box[0] = i
                if p not in loaded:
                    loaded[p] = load_pair(p)
                if qt == 1 and j == 0 and p + 1 < PPC and p + 1 not in loaded:
                    loaded[p + 1] = load_pair(p + 1)
                for old in [k for k in loaded if k < p - 1]:
                    del loaded[old]
                if p == 0 and j == 0:
                    want_bias(qt + 1)
                if p == 0 and j == KP // 2:
                    want_bias(qt + 2)
                if (p, qt) not in state:
                    o_acc = acc_pool.tile(
                        [D + 1, 2, QTILE], f32, name="oacc", tag="oacc"
                    )
                    state[(p, qt)] = {
                        "o_lo": o_acc[:, 0, :],
                        "o_hi": o_acc[:, 1, :],
                        "o_acc": o_acc,
                    }
                p_sb = produce(p, qt, j)
                if len(pending) >= LAG:
                    pp, pq, pj, ppsb, plast = pending.pop(0)
                    consume(pp, pq, pj, ppsb)
                    if plast:
                        epilogue_start(pp, pq)
                epi_step()
                pending.append((p, qt, j, p_sb, is_last))
            while pending:
                iter_box[0] += 1
                pp, pq, pj, ppsb, plast = pending.pop(0)
                consume(pp, pq, pj, ppsb)
                if plast:
                    epilogue_start(pp, pq)
                epi_step()
            while epi_steps:
                iter_box[0] += 1
                epi_step()

    return nc


def _get_nc():
    if "nc" not in _CACHE:
        nc = _build_nc()
        nc.finalize()
        _CACHE["nc"] = nc
    return _CACHE["nc"]


def _make_in_maps(mat1, mat2, mat3, bias):
    import ml_dtypes

    bf16 = ml_dtypes.bfloat16
    q = np.asarray(mat1, dtype=np.float32).reshape(PAIRS, S, D) * SCALE
    k = np.asarray(mat2, dtype=np.float32).reshape(PAIRS, S, D)
    v = np.asarray(mat3, dtype=np.float32).reshape(PAIRS, S, D)
    v1 = np.concatenate([v, np.ones((PAIRS, S, 1), np.float32)], axis=2)
    # pre-arrange to the on-chip layout [pair, partition, ktile, d]
    v1 = v1.reshape(PAIRS, KT, 128, D + 1).transpose(0, 2, 1, 3)
    v1 = np.ascontiguousarray(v1)

    # qd: [PAIRS, 128, S] -- qT duplicated on both partition halves
    qT = q.transpose(0, 2, 1)
    qdup = np.ascontiguousarray(np.concatenate([qT, qT], axis=1).astype(bf16))

    # kp: [PAIRS, 128, KP, 128]: rows 0-63 = kT of even k-tiles, rows
    # 64-127 = kT of odd k-tiles.
    kT = k.transpose(0, 2, 1).reshape(PAIRS, D, KP, 2, 128)  # [P, D, j, par, c]
    k_even = kT[:, :, :, 0, :]                      # [P, D, KP, 128]
    k_odd = kT[:, :, :, 1, :]
    kpack = np.concatenate([k_even, k_odd], axis=1)  # [P, 128, KP, 128]
    kpack = np.ascontiguousarray(kpack.astype(bf16))

    biasT_f = np.asarray(bias, dtype=np.float32).reshape(S, S).T
    biasT = np.ascontiguousarray(biasT_f.astype(bf16))
    ident = np.eye(128, dtype=np.float32).astype(bf16)

    # partition-swapped bias rows for PE-path chunks (see _build_nc)
    pe_path = _pe_pattern()
    pe_js = [j for j in range(KP) if pe_path[j]]
    bias_x = None
    if pe_js:
        bx = np.empty((128, len(pe_js), S), np.float32)
        for c, j in enumerate(pe_js):
            bx[0:64, c, :] = biasT_f[2 * j * 128 + 64 : 2 * j * 128 + 128, :]
            bx[64:128, c, :] = biasT_f[(2 * j + 1) * 128 : (2 * j + 1) * 128 + 64, :]
        bias_x = np.ascontiguousarray(bx.astype(bf16))

    in_maps = []
    for c in range(N_CORES):
        sl = slice(c * PPC, (c + 1) * PPC)
        m = {
            "qd": qdup[sl],
            "kp": kpack[sl],
            "v1": np.ascontiguousarray(v1[sl].astype(bf16)),
            "biasT": biasT,
            "ident": ident,
        }
        if bias_x is not None:
            m["bias_x"] = bias_x
        in_maps.append(m)
    return in_maps


def kernel(mat1, mat2, mat3, bias):
    from concourse.bass_utils import run_bass_kernel_spmd

    in_maps = _make_in_maps(mat1, mat2, mat3, bias)
    nc = _get_nc()
    _CACHE["in_maps"] = in_maps
    res = run_bass_kernel_spmd(nc, in_maps, list(range(N_CORES)))
    outs = [res.results[c]["out"] for c in range(N_CORES)]
    full = np.concatenate(outs, axis=0).reshape(B, H, S, D)
    return full.astype(np.float32)


# revision 31
# speedup vs baseline: 1.1798x; 1.1798x over previous
"""MHA forward kernel for Trainium2 (Bass/Tile), sharded over (batch, head)
pairs across 8 NeuronCores.

Math (per (b,h) pair):
    scores = softmax(Q K^T / sqrt(64) + bias)   # bias broadcast over (b,h)
    out    = scores @ V

The whole matmul pipeline runs in the PE's 64x128 row-tiled mode
(tile_position (0,0) / (64,0)), exploiting the D=64 contraction:
  - MM1 computes TWO score k-tiles concurrently (even k-tiles through array
    rows 0-63, odd through rows 64-127), doubling QK^T throughput.
  - bias is added into PSUM via identity matmuls for N_PE of the 8 chunks
    per q-tile (PE-path), via DVE scalar_tensor_tensor for the rest
    (DVE-path) -- the split balances PE vs DVE vs ACT occupancy. Each PSUM
    bank is only ever written by its own PE tile (two tiles writing one
    bank wedges the device); the cross-half bias rows come from bias_x, a
    partition-swapped host-side copy, through sident (a swapped identity).
  - MM2 accumulates the lower 64 rows of every k-tile into o_lo (tile T0)
    and the upper rows into o_hi (T8) concurrently; the epilogue reduces
    them, PE-transposes 128-row blocks, and divides by the ones-column sum.
  - epilogue transpose/div/store steps are deferred and spread one per
    stream iteration so their DVE latency never head-of-line-blocks the
    in-order PE queue; transpose PSUM targets share the score pool's ring.
  - bias q-slices stream in lazily (2 q-tiles ahead) to keep the startup
    DMA window clear for the first pair's Q/K/V.
Q is pre-scaled by 1/8 host-side (exact in bf16) and duplicated onto both
partition halves; K is host-packed even/odd so each PE half-array reads its
stationary tiles from its own SBUF partitions.
"""

import os
import sys

import numpy as np

for _p in ("/opt/trn_rl_repo",):
    if _p not in sys.path and os.path.isdir(_p):
        sys.path.insert(0, _p)

B, H, S, D = 2, 16, 2048, 64
N_CORES = 8
PAIRS = B * H                     # 32
PPC = PAIRS // N_CORES            # 4 pairs per core
SCALE = 1.0 / 8.0                 # 1/sqrt(64)

KT = S // 128                     # 16 k-tiles of 128
KP = KT // 2                      # 8 k-tile pairs (even/odd)
QTILE = 512
QT = S // QTILE                   # 4 q-tiles
PP_BUFS = int(os.environ.get("PP_BUFS", "4"))
PD_BUFS = int(os.environ.get("PD_BUFS", "4"))
SC_BUFS = int(os.environ.get("SC_BUFS", "3"))
LAG = int(os.environ.get("LAG", "3"))
EPI_BUFS = int(os.environ.get("EPI_BUFS", "3"))
N_PE = int(os.environ.get("N_PE", "4"))

_CACHE = {}


def _pe_pattern():
    """Which of the 8 chunks per q-tile add bias on the PE (identity
    matmuls) vs the DVE (scalar_tensor_tensor)."""
    pat = [False] * KP
    if N_PE >= KP:
        return [True] * KP
    # grouped pairs: PE,PE,DVE,DVE,... so adjacent DVE chunks can share
    # one batched exp instruction
    order = [0, 1, 4, 5, 2, 3, 6, 7]
    for i in range(N_PE):
        pat[order[i]] = True
    return pat


def _build_nc():
    import concourse.mybir as mybir
    import concourse.tile as tile
    from concourse import bacc

    f32 = mybir.dt.float32
    bf16 = mybir.dt.bfloat16
    nc = bacc.Bacc(None)

    qd = nc.declare_dram_parameter("qd", [PPC, 128, S], bf16, isOutput=False)
    kp = nc.declare_dram_parameter("kp", [PPC, 128, KP, 128], bf16, isOutput=False)
    v1 = nc.declare_dram_parameter("v1", [PPC, 128, KT, D + 1], bf16, isOutput=False)
    biasT = nc.declare_dram_parameter("biasT", [S, S], bf16, isOutput=False)
    ident_d = nc.declare_dram_parameter("ident", [128, 128], bf16, isOutput=False)
    out = nc.declare_dram_parameter("out", [PPC, S, D], bf16, isOutput=True)

    pe_path = _pe_pattern()
    n_pe = sum(pe_path)
    # bias_x holds the partition-swapped bias rows for PE-path chunks so
    # each PSUM bank's bias is added by the SAME tile that wrote its
    # scores (two tiles writing one bank hangs the device):
    #   partitions 0-63:  rows 64-127 of each PE-chunk's even k-tile
    #   partitions 64-127: rows 0-63 of each PE-chunk's odd k-tile
    if n_pe:
        bias_x = nc.declare_dram_parameter(
            "bias_x", [128, n_pe, S], bf16, isOutput=False
        )

    with tile.TileContext(nc) as tc:
        with (
            tc.tile_pool(name="const", bufs=1) as const_pool,
            tc.tile_pool(name="bias", bufs=1) as bias_pool,
            tc.tile_pool(name="qk", bufs=2) as qk_pool,
            tc.tile_pool(name="vv", bufs=2) as v_pool,
            tc.tile_pool(name="probP", bufs=PP_BUFS) as pP_pool,
            tc.tile_pool(name="probD", bufs=PD_BUFS) as pD_pool,
            tc.tile_pool(name="epi", bufs=EPI_BUFS) as epi_pool,
            tc.tile_pool(name="sc", bufs=SC_BUFS, space="PSUM") as sc_pool,
            tc.tile_pool(name="acc", bufs=1, space="PSUM") as acc_pool,
        ):
            ident = const_pool.tile([128, 128], bf16)
            nc.sync.dma_start(ident[:], ident_d[:])
            # Dummy matmuls bridge the startup DMA window: they keep the
            # PE HAM activity monitor at full clock (idle >3.4us would
            # re-throttle to 1.2GHz) and cost nothing -- the PE would be
            # idle waiting for Q/K/bias anyway. Output is never read.
            warm = sc_pool.tile([128, 2, QTILE], f32, tag="sc", name="warm")
            for w in range(48):
                nc.tensor.matmul(
                    warm[:, 0, 0:128], ident[0:64, :], ident[0:64, :],
                    start=(w == 0), stop=(w == 47), tile_position=(0, 0),
                )
            # partition-swapped identity: rows 0-63 hold e_{64+r}, rows
            # 64-127 hold e_r -- lets each tile add bias into the
            # opposite 64-row half of its own PSUM bank.
            sident = const_pool.tile([128, 128], bf16)
            nc.sync.dma_start(sident[0:64, :], ident_d[64:128, :])
            nc.sync.dma_start(sident[64:128, :], ident_d[0:64, :])

            def load_pair(p):
                qd_sb = qk_pool.tile([128, S], bf16, tag="q")
                nc.sync.dma_start(qd_sb[:], qd[p])
                kp_sb = qk_pool.tile([128, KP, 128], bf16, tag="k")
                nc.sync.dma_start(kp_sb[:], kp[p])
                # V with ones-column appended (host-side): second matmul
                # also yields sum(exp) in output row D.
                # host pre-arranged to the SBUF layout: the DMA moves
                # one contiguous 2080B run per partition instead of 16
                # 130B runs (16x fewer descriptors -- less startup DMA
                # engine congestion)
                v_sb = v_pool.tile([128, KT, D + 1], bf16)
                nc.sync.dma_start(v_sb[:], v1[p])
                return qd_sb, kp_sb, v_sb

            # pair 0 loads first so MM1 can start before the bias stream.
            loaded = {0: load_pair(0)}

            # Full bias^T resident in SBUF: [128, KT, S] bf16. Only the
            # qt=0 slice is loaded upfront (per-ktile, chunk order);
            # later q-slices are issued lazily from the stream loop so
            # the startup DMA window isn't congested by 10MB of bias.
            bias_sb = bias_pool.tile([128, KT, S], bf16)
            bias_src = biasT.rearrange("(kt p) q -> p kt q", p=128)
            if n_pe:
                bias_x_sb = bias_pool.tile([128, n_pe, S], bf16, tag="bx")

            def load_bias_slice(qc, fine):
                qsl = slice(qc * QTILE, (qc + 1) * QTILE)
                if n_pe:
                    nc.sync.dma_start(
                        bias_x_sb[:, :, qsl], bias_x[:, :, qsl]
                    )
                if fine:
                    for kt in range(KT):
                        nc.sync.dma_start(
                            bias_sb[:, kt, qsl], bias_src[:, kt, qsl]
                        )
                else:
                    nc.sync.dma_start(
                        bias_sb[:, :, qsl], bias_src[:, :, qsl]
                    )

            load_bias_slice(0, fine=True)

            pe_idx = {}
            for j in range(KP):
                if pe_path[j]:
                    pe_idx[j] = len(pe_idx)

            stream = []  # (p, qt, j, is_last_of_qt)
            for p in range(PPC):
                for qt in range(QT):
                    for j in range(KP):
                        stream.append((p, qt, j, j == KP - 1))

            state = {}  # (p, qt) -> dict with o_ev, o_od
            dve_hold = [None]

            def produce(p, qt, j):
                qd_sb, kp_sb, _ = loaded[p]
                qs_lo = qd_sb[0:64, qt * QTILE : (qt + 1) * QTILE]
                qs_hi = qd_sb[64:128, qt * QTILE : (qt + 1) * QTILE]
                kt_e, kt_o = 2 * j, 2 * j + 1
                qsl = slice(qt * QTILE, (qt + 1) * QTILE)
                on_pe = pe_path[j]
                s = sc_pool.tile([128, 2, QTILE], f32, tag="sc")
                # scores: even k-tile through rows 0-63 (T0), odd through
                # rows 64-127 (T8) -- concurrent in 64x128 tiled mode.
                nc.tensor.matmul(
                    s[:, 0, :], kp_sb[0:64, j, :], qs_lo,
                    start=True, stop=not on_pe, tile_position=(0, 0),
                )
                nc.tensor.matmul(
                    s[:, 1, :], kp_sb[64:128, j, :], qs_hi,
                    start=True, stop=not on_pe, tile_position=(64, 0),
                )
                if on_pe:
                    # bias via identity matmuls, every bank written ONLY
                    # by its own tile: T0 owns bank0 (even k-tile), T8
                    # owns bank1 (odd). The cross-half rows come from
                    # bias_x (partition-swapped copy) through sident.
                    c = pe_idx[j]
                    nc.tensor.matmul(
                        s[:, 0, :], ident[0:64, :], bias_sb[0:64, kt_e, qsl],
                        start=False, stop=False, tile_position=(0, 0),
                    )
                    nc.tensor.matmul(
                        s[:, 1, :], ident[64:128, :], bias_sb[64:128, kt_o, qsl],
                        start=False, stop=False, tile_position=(64, 0),
                    )
                    nc.tensor.matmul(
                        s[:, 0, :], sident[0:64, :], bias_x_sb[0:64, c, qsl],
                        start=False, stop=True, tile_position=(0, 0),
                    )
                    nc.tensor.matmul(
                        s[:, 1, :], sident[64:128, :], bias_x_sb[64:128, c, qsl],
                        start=False, stop=True, tile_position=(64, 0),
                    )
                    p_sb = pP_pool.tile([128, 2, QTILE], bf16, tag="pP")
                    nc.scalar.activation(
                        p_sb[:], s[:], mybir.ActivationFunctionType.Exp
                    )
                    return (p_sb, 0)
                p_sb = pD_pool.tile([128, 2, QTILE], bf16, tag="pD")
                nc.vector.scalar_tensor_tensor(
                    p_sb[:],
                    s[:],
                    1.0,
                    bias_sb[:, kt_e : kt_e + 2, qsl],
                    op0=mybir.AluOpType.mult,
                    op1=mybir.AluOpType.add,
                )
                nc.scalar.activation(
                    p_sb[:], p_sb[:], mybir.ActivationFunctionType.Exp
                )
                return (p_sb, 0)

            def consume(p, qt, j, p_sb_ref):
                p_sb, base = p_sb_ref
                _, _, v_sb = loaded[p]
                st = state[(p, qt)]
                kt_e, kt_o = 2 * j, 2 * j + 1
                # T0 accumulates the lower 64 rows of every k-tile into
                # o_lo; T8 the upper 64 rows into o_hi -- concurrent.
                nc.tensor.matmul(
                    st["o_lo"],
                    v_sb[0:64, kt_e, :],
                    p_sb[0:64, base + 0, :],
                    start=(j == 0), stop=False,
                    tile_position=(0, 0),
                )
                nc.tensor.matmul(
                    st["o_hi"],
                    v_sb[64:128, kt_e, :],
                    p_sb[64:128, base + 0, :],
                    start=(j == 0), stop=False,
                    tile_position=(64, 0),
                )
                nc.tensor.matmul(
                    st["o_lo"],
                    v_sb[0:64, kt_o, :],
                    p_sb[0:64, base + 1, :],
                    start=False, stop=(j == KP - 1),
                    tile_position=(0, 0),
                )
                nc.tensor.matmul(
                    st["o_hi"],
                    v_sb[64:128, kt_o, :],
                    p_sb[64:128, base + 1, :],
                    start=False, stop=(j == KP - 1),
                    tile_position=(64, 0),
                )

            epi_steps = []  # deferred per-128-row epilogue work items
            iter_box = [0]

            def epilogue_start(p, qt):
                # Only the PSUM->SBUF reduce runs immediately (frees
                # o_acc); the transpose/div/store steps are spread one
                # per stream iteration so their DVE latency never
                # head-of-line-blocks the in-order PE queue.
                st = state.pop((p, qt))
                # bf16 here halves the PE transpose cost (1 cyc/row vs 2
                # for fp32); the 2-element add + later divide tolerate it
                o_sb = epi_pool.tile([D + 1, QTILE], bf16, tag="osb")
                with nc.allow_low_precision("bf16 epilogue, tol 2e-2"):
                    nc.vector.tensor_reduce(
                        o_sb[:],
                        st["o_acc"][:].rearrange("p a b -> p b a"),
                        axis=mybir.AxisListType.X,
                        op=mybir.AluOpType.add,
                    )
                for c in range(4):
                    # first transpose waits the DVE reduce (~1.2us); hold
                    # it back two stream slots so the in-order PE queue
                    # keeps streaming matmuls meanwhile
                    epi_steps.append((iter_box[0] + 2 + c, p, qt, c, o_sb))

            def epi_step():
                if not epi_steps or epi_steps[0][0] > iter_box[0]:
                    return
                _, p, qt, c, o_sb = epi_steps.pop(0)
                # transpose target shares the score pool's ring (same
                # shape/tag keeps one 3-buf ring within 6 PSUM banks)
                t_tile = sc_pool.tile([128, 2, QTILE], bf16, tag="sc")
                t_psum = t_tile[:, 0, : D + 1]
                nc.tensor.transpose(
                    t_psum,
                    o_sb[:, c * 128 : (c + 1) * 128],
                    ident[: D + 1, : D + 1],
                )
                r_sb = epi_pool.tile([128, 1], f32, tag="rsb")
                nc.vector.reciprocal(r_sb[:], t_tile[:, 0, D : D + 1])
                f_sb = epi_pool.tile([128, D], bf16, tag="fsb")
                nc.vector.tensor_scalar_mul(f_sb[:], t_tile[:, 0, :D], r_sb[:])
                row0 = qt * QTILE + c * 128
                nc.sync.dma_start(out[p, row0 : row0 + 128, :], f_sb[:])

            bias_loaded = {0}

            def want_bias(qc):
                if 0 <= qc < QT and qc not in bias_loaded:
                    bias_loaded.add(qc)
                    load_bias_slice(qc, fine=False)

            pending = []  # (p, qt, j, p_sb, is_last)
            for i, (p, qt, j, is_last) in enumerate(stream):
                iter_box[0] = i
                if p not in loaded:
                    loaded[p] = load_pair(p)
                # DMA issue order matches need order: bias(qt+1) first,
                # then the next pair's Q/K/V, then bias(qt+2) -- queues
                # drain FIFO, so later-needed bias must not sit ahead of
                # the next pair's inputs.
                if p == 0 and j == 0:
                    want_bias(qt + 1)
                if qt == 0 and j == KP - 2 and p + 1 < PPC and p + 1 not in loaded:
                    loaded[p + 1] = load_pair(p + 1)
                for old in [k for k in loaded if k < p - 1]:
                    del loaded[old]
                if p == 0 and j == KP - 1:
                    want_bias(qt + 2)
                if (p, qt) not in state:
                    o_acc = acc_pool.tile(
                        [D + 1, 2, QTILE], f32, name="oacc", tag="oacc"
                    )
                    state[(p, qt)] = {
                        "o_lo": o_acc[:, 0, :],
                        "o_hi": o_acc[:, 1, :],
                        "o_acc": o_acc,
                    }
                p_sb = produce(p, qt, j)
                if len(pending) >= LAG:
                    pp, pq, pj, ppsb, plast = pending.pop(0)
                    consume(pp, pq, pj, ppsb)
                    if plast:
                        epilogue_start(pp, pq)
                epi_step()
                pending.append((p, qt, j, p_sb, is_last))
            while pending:
                iter_box[0] += 1
                pp, pq, pj, ppsb, plast = pending.pop(0)
                consume(pp, pq, pj, ppsb)
                if plast:
                    epilogue_start(pp, pq)
                epi_step()
            while epi_steps:
                iter_box[0] += 1
                epi_step()

    return nc


def _get_nc():
    if "nc" not in _CACHE:
        nc = _build_nc()
        nc.finalize()
        _CACHE["nc"] = nc
    return _CACHE["nc"]


def _make_in_maps(mat1, mat2, mat3, bias):
    import ml_dtypes

    bf16 = ml_dtypes.bfloat16
    q = np.asarray(mat1, dtype=np.float32).reshape(PAIRS, S, D) * SCALE
    k = np.asarray(mat2, dtype=np.float32).reshape(PAIRS, S, D)
    v = np.asarray(mat3, dtype=np.float32).reshape(PAIRS, S, D)
    v1 = np.concatenate([v, np.ones((PAIRS, S, 1), np.float32)], axis=2)
    # pre-arrange to the on-chip layout [pair, partition, ktile, d]
    v1 = v1.reshape(PAIRS, KT, 128, D + 1).transpose(0, 2, 1, 3)
    v1 = np.ascontiguousarray(v1)

    # qd: [PAIRS, 128, S] -- qT duplicated on both partition halves
    qT = q.transpose(0, 2, 1)
    qdup = np.ascontiguousarray(np.concatenate([qT, qT], axis=1).astype(bf16))

    # kp: [PAIRS, 128, KP, 128]: rows 0-63 = kT of even k-tiles, rows
    # 64-127 = kT of odd k-tiles.
    kT = k.transpose(0, 2, 1).reshape(PAIRS, D, KP, 2, 128)  # [P, D, j, par, c]
    k_even = kT[:, :, :, 0, :]                      # [P, D, KP, 128]
    k_odd = kT[:, :, :, 1, :]
    kpack = np.concatenate([k_even, k_odd], axis=1)  # [P, 128, KP, 128]
    kpack = np.ascontiguousarray(kpack.astype(bf16))

    biasT_f = np.asarray(bias, dtype=np.float32).reshape(S, S).T
    biasT = np.ascontiguousarray(biasT_f.astype(bf16))
    ident = np.eye(128, dtype=np.float32).astype(bf16)

    # partition-swapped bias rows for PE-path chunks (see _build_nc)
    pe_path = _pe_pattern()
    pe_js = [j for j in range(KP) if pe_path[j]]
    bias_x = None
    if pe_js:
        bx = np.empty((128, len(pe_js), S), np.float32)
        for c, j in enumerate(pe_js):
            bx[0:64, c, :] = biasT_f[2 * j * 128 + 64 : 2 * j * 128 + 128, :]
            bx[64:128, c, :] = biasT_f[(2 * j + 1) * 128 : (2 * j + 1) * 128 + 64, :]
        bias_x = np.ascontiguousarray(bx.astype(bf16))

    in_maps = []
    for c in range(N_CORES):
        sl = slice(c * PPC, (c + 1) * PPC)
        m = {
            "qd": qdup[sl],
            "kp": kpack[sl],
            "v1": np.ascontiguousarray(v1[sl].astype(bf16)),
            "biasT": biasT,
            "ident": ident,
        }
        if bias_x is not None:
            m["bias_x"] = bias_x
        in_maps.append(m)
    return in_maps


def kernel(mat1, mat2, mat3, bias):
    from concourse.bass_utils import run_bass_kernel_spmd

    in_maps = _make_in_maps(mat1, mat2, mat3, bias)
    nc = _get_nc()
    _CACHE["in_maps"] = in_maps
    res = run_bass_kernel_spmd(nc, in_maps, list(range(N_CORES)))
    outs = [res.results[c]["out"] for c in range(N_CORES)]
    full = np.concatenate(outs, axis=0).reshape(B, H, S, D)
    return full.astype(np.float32)
